# revision 30
# baseline (speedup 1.0000x reference)
"""Trainium2 Bass kernel for the NMS-BP decoder — PE-routed (bf16-triple) edition.

Self-contained: takes the FULL inputs of reference.setup_inputs(), shards the
batch across 8 NeuronCores (pure data parallelism), runs a Bass/Tile NEFF per
core, and reassembles the full [6, 64, 1024] output.

Per core (B_local = 8) the whole decoder lives in SBUF/PSUM. The two sparse
routings per iteration (column sums -> slots, slots -> column sums) run on
the TENSOR engine as one-hot matmuls:

  * weights = 0/1 incidence tiles in fp8e4 (exact), DMA'd in consumption-order
    chunks (wc before wg) so compute never waits on the big weight transfers;
  * moving data = bf16 TRIPLE (h, m, l) packed in the free dim: x = h+m+l
    reconstructs fp32 bit-exactly (3x8 significand bits), and bf16 keeps the
    full fp32 exponent range so no component goes subnormal (fp16 pairs hit
    the PE denormal path at ~100x cost);
  * PSUM accumulates in fp32; every product is 1.0 * bf16 so routing is exact.

The gather routes the column-sum cs only — temp = cs + c1 never materializes:
g_c1 = (sp1*soft)[cols] is precomputed EXACTLY (fp32) on the host and DMA'd
in, and the reconstruct adds qm = g_c1 - cv (GpSimd, off the critical chain).
Iteration 1 (cv = 0, cs = 0) therefore needs no gather at all: vc(1) = g_c1,
so the DVE chain starts as soon as the small g_c1 DMA lands instead of after
the ~3.4 MB weight load.  The colsum runs as two k-halves with separate PSUM
banks and per-half cs/split chains, letting the next iteration's gather start
on first-half chunks while the second half is still summing.

Checks are reassigned to (mhi, mlow) positions sorted by mean column index,
which concentrates each q-plane's columns into few 128-column chunks: only
~104 of 192 (q, k) incidence tiles are nonzero and empty tiles are skipped.

The 12-comparator 6-lane sorting network runs as 5 fused layers (13 wide DVE
ops instead of 24 narrow ones); physical j-planes hold logical edge lanes in
LOGMAP order so layer 1 is a contiguous half-vs-half min/max. abs/sign run as
single Activation-engine ops, the sign products and sign*psign on GpSimd, and
the w_k scalings as scaled Activation copies, so DVE keeps only the critical
chain.

Layouts:
  check/slot domain: [128 p = mlow, 24 q = jp*4 + mhi, 8 b]; slot s = q*128+p,
  col(s) = row_cols[assign[(q%4)*128 + p], LOGMAP[q//4]].
  column domain:     [128 p = nlow, 8 k, 8 b]; column n = k*128 + p.
"""

import numpy as np

B, N, M, DC, NUM_ITERS = 64, 1024, 512, 6, 5
NCORES = 8
BL = B // NCORES          # 8 batch rows per core
NSLOT = M * DC            # 3072
LOGMAP = [0, 1, 2, 5, 3, 4]   # physical j-plane -> logical (sorted-col) lane
WG_CHUNKS = 8
WC_CHUNKS = 4


def _chunk_bounds(n, nchunks):
    base, rem = divmod(n, nchunks)
    bounds = [0]
    for i in range(nchunks):
        bounds.append(bounds[-1] + base + (1 if i < rem else 0))
    return bounds

_CACHE = {}


def _layout(row_cols):
    """Check assignment (sorted by mean col) + per-slot columns + tile lists."""
    assign = np.argsort(row_cols.mean(axis=1), kind="stable")  # position -> check
    cols = np.empty(NSLOT, np.int64)
    for q in range(24):
        jp, mhi = q // 4, q % 4
        j = LOGMAP[jp]
        for p in range(128):
            cols[q * 128 + p] = row_cols[assign[mhi * 128 + p], j]
    present = [sorted({int(c) // 128 for c in cols[q * 128:(q + 1) * 128]})
               for q in range(24)]
    gt = [(q, k) for q in range(24) for k in present[q]]           # gather tiles
    ct = [(k, q) for k in range(8) for q in range(24) if k in present[q]]
    return assign, cols, present, gt, ct


def _weights(cols, gt, ct):
    wg = np.zeros((128, len(gt), 128), np.float32)
    for t, (q, k) in enumerate(gt):
        for po in range(128):
            c = cols[q * 128 + po]
            if c // 128 == k:
                wg[c % 128, t, po] = 1.0
    wc = np.zeros((128, len(ct), 128), np.float32)
    for t, (k, q) in enumerate(ct):
        for ps in range(128):
            c = cols[q * 128 + ps]
            if c // 128 == k:
                wc[ps, t, c % 128] = 1.0
    return wg, wc


def _build(cols, w, sp1, sp2, gt, ct):
    import concourse.bass as bass
    import concourse.bacc as bacc
    import concourse.tile as tile
    import concourse.mybir as mybir

    dt = mybir.dt
    Alu = mybir.AluOpType
    ActF = mybir.ActivationFunctionType
    f32 = dt.float32
    bf16 = dt.bfloat16
    f8 = dt.float8e4

    nc = bacc.Bacc("TRN2", target_bir_lowering=False, debug=False)

    NGT, NCT = len(gt), len(ct)
    soft_t = nc.dram_tensor("soft_t", [N, BL], f32, kind="ExternalInput")
    gc1_d = nc.dram_tensor("gc1", [128, 24 * BL], f32, kind="ExternalInput")
    wg_d = nc.dram_tensor("wg", [128, NGT * 128], f8, kind="ExternalInput")
    wc_d = nc.dram_tensor("wc", [128, NCT * 128], f8, kind="ExternalInput")

    gb = _chunk_bounds(NGT, WG_CHUNKS)
    cb = _chunk_bounds(NCT, WC_CHUNKS)

    def g_chunk(t):
        for i in range(WG_CHUNKS):
            if gb[i] <= t < gb[i + 1]:
                return i, t - gb[i]
        raise AssertionError

    def c_chunk(t):
        for i in range(WC_CHUNKS):
            if cb[i] <= t < cb[i + 1]:
                return i, t - cb[i]
        raise AssertionError
    out = nc.dram_tensor("out", [NUM_ITERS + 1, N, BL], f32, kind="ExternalOutput")

    w = [float(x) for x in w]
    sp1 = float(sp1)
    sp2 = float(sp2)

    gt_pos = {qk: t for t, qk in enumerate(gt)}
    ct_pos = {kq: t for t, kq in enumerate(ct)}
    pres_q = {}
    for (q, k) in gt:
        pres_q.setdefault(q, []).append(k)
    pres_k = {}
    for (k, q) in ct:
        pres_k.setdefault(k, []).append(q)

    with tile.TileContext(nc) as tc:
        with (
            tc.tile_pool(name="const", bufs=1) as pc,
            tc.tile_pool(name="work", bufs=2) as pw,
            tc.tile_pool(name="srt", bufs=12) as psrt,
            tc.tile_pool(name="small", bufs=24) as psm,
            tc.tile_pool(name="ppg", bufs=1, space="PSUM") as ppg,
            tc.tile_pool(name="ppc", bufs=1, space="PSUM") as ppc,
        ):
            sT = pc.tile([128, 8, BL], f32)
            nc.sync.dma_start(sT[:, :, :], soft_t.rearrange("(nh p) b -> p nh b", p=128))
            gc1 = pc.tile([128, 24, BL], f32)
            nc.sync.dma_start(gc1[:, :, :].rearrange("p q b -> p (q b)"), gc1_d[:, :])
            nc.sync.dma_start(out[0][:, :], soft_t[:, :])
            wc_sb = []
            for i in range(WC_CHUNKS):
                sz = cb[i + 1] - cb[i]
                t_ = pc.tile([128, sz, 128], f8, tag=f"wc{i}", name=f"wc{i}")
                nc.sync.dma_start(
                    t_[:, :, :].rearrange("p a c -> p (a c)"),
                    wc_d[:, cb[i] * 128:cb[i + 1] * 128])
                wc_sb.append(t_)
            wg_sb = []
            for i in range(WG_CHUNKS):
                sz = gb[i + 1] - gb[i]
                t_ = pc.tile([128, sz, 128], f8, tag=f"wg{i}", name=f"wg{i}")
                nc.sync.dma_start(
                    t_[:, :, :].rearrange("p a c -> p (a c)"),
                    wg_d[:, gb[i] * 128:gb[i + 1] * 128])
                wg_sb.append(t_)
            c2 = pc.tile([128, 8, BL], f32)
            nc.any.tensor_scalar(c2[:, :, :], sT[:, :, :], sp2, None, Alu.mult)

            _split_ctr = [0]

            def split_tri(src_f32, tri, nmid):
                """tri[:, :, 0..2, :] = bf16 triple of src (h, m, l); mixed-dtype
                subtracts skip the f32 upcast copies."""
                _split_ctr[0] = (_split_ctr[0] + 1) % 8
                nc.vector.tensor_copy(tri[:, :, 0, :], src_f32)
                r = pw.tile([128, nmid, BL], f32, tag=f"r{nmid}_{_split_ctr[0]}", name="r")
                nc.vector.tensor_tensor(r[:, :, :], src_f32, tri[:, :, 0, :], Alu.subtract)
                nc.vector.tensor_copy(tri[:, :, 1, :], r[:, :, :])
                nc.vector.tensor_tensor(tri[:, :, 2, :], r[:, :, :], tri[:, :, 1, :], Alu.subtract)

            def do_gather(temp_tri):
                vcp = [ppg.tile([128, 6, 3, BL], f32, tag=f"vc{c}", name=f"vc{c}")
                       for c in range(4)]
                for q in range(24):
                    o = vcp[q // 6][:, q % 6, :, :].rearrange("p t b -> p (t b)")
                    ks = pres_q[q]
                    for i, k in enumerate(ks):
                        ci, off = g_chunk(gt_pos[(q, k)])
                        nc.tensor.matmul(
                            o, wg_sb[ci][:, off, :],
                            cs_tri[k // 2][:, k % 2, :, :].rearrange("p t b -> p (t b)"),
                            start=(i == 0), stop=(i == len(ks) - 1))
                return vcp

            def do_colsum_half(cv_tri, h):
                cs_ps = ppc.tile([128, 2, 3, BL], f32, tag=f"csps{h}", name="cs_ps")
                for kk in range(2):
                    k = 2 * h + kk
                    o = cs_ps[:, kk, :, :].rearrange("p t b -> p (t b)")
                    qs = sorted(pres_k[k], key=lambda q: (q >= 12, q))
                    for i, q in enumerate(qs):
                        ci, off = c_chunk(ct_pos[(k, q)])
                        nc.tensor.matmul(
                            o, wc_sb[ci][:, off, :],
                            cv_tri[:, q, :, :].rearrange("p t b -> p (t b)"),
                            start=(i == 0), stop=(i == len(qs) - 1))
                return cs_ps

            def pl(t, i, n=1):
                """n plane-groups of 4 starting at plane i."""
                return t[:, 4 * i:4 * (i + n), :]

            def g3(t, gidx):
                """planes (gidx, gidx+3) as [128, 2, 4, BL] (stride-3 pair)."""
                return t[:, :, :].rearrange("p (two g m) b -> p two g m b", two=2, g=3)[:, :, gidx, :, :]

            def w2(t, i):
                """planes (i, i+2) as [128, 2, 4, BL] (stride-2 pair window)."""
                return t[:, 4 * i:4 * i + 16, :].rearrange(
                    "p (two g m) b -> p two g m b", two=2, g=2)[:, :, 0, :, :]

            cv = None
            cs_tri = None
            qm = None
            for it in range(1, NUM_ITERS + 1):
                if it == 1:
                    vc = gc1  # vc(1) = (sp1*soft)[cols], exact, host-gathered
                else:
                    vcp = do_gather(cs_tri)
                    vc = pw.tile([128, 24, BL], f32, tag="vc", name="vc")
                    for ch in (0, 2, 1, 3):
                        sl = slice(6 * ch, 6 * ch + 6)
                        ps_t = vcp[ch]
                        g1 = pw.tile([128, 6, BL], f32, tag=f"g1h{ch}", name="g1")
                        nc.vector.tensor_tensor(g1[:, :, :], ps_t[:, :, 0, :], qm[:, sl, :], Alu.add)
                        g2 = pw.tile([128, 6, BL], f32, tag=f"g2h{ch}", name="g2")
                        nc.vector.tensor_tensor(g2[:, :, :], g1[:, :, :], ps_t[:, :, 1, :], Alu.add)
                        nc.vector.tensor_tensor(vc[:, sl, :], g2[:, :, :], ps_t[:, :, 2, :], Alu.add)

                # ---- vector phase. abs runs in interleaved halves that
                # match sort layer 1's operand pairs ({0-5,12-17} then
                # {6-11,18-23}), so L1a starts before the last gather chunk ----
                a = pw.tile([128, 24, BL], f32, tag="a")

                def ihalf(t, lo):
                    return t[:, :, :].rearrange(
                        "p (two g) b -> p two g b", two=2)[:, :, lo:lo + 6, :]

                nc.scalar.activation(ihalf(a, 0), ihalf(vc, 0), ActF.Abs)
                nc.scalar.activation(ihalf(a, 6), ihalf(vc, 6), ActF.Abs)
                sg = pw.tile([128, 24, BL], f32, tag="sg")
                nc.scalar.activation(sg[:, :, :], vc[:, :, :], ActF.Sign)

                # psign on gpsimd (parallel with DVE sort)
                p1 = psm.tile([128, 12, BL], f32, tag="p1")
                nc.gpsimd.tensor_tensor(p1[:, :, :], sg[:, 0:12, :], sg[:, 12:24, :], Alu.mult)
                p2 = psm.tile([128, 4, BL], f32, tag="p2")
                nc.gpsimd.tensor_tensor(p2[:, :, :], p1[:, 0:4, :], p1[:, 4:8, :], Alu.mult)
                ps = psm.tile([128, 4, BL], f32, tag="ps")
                nc.gpsimd.tensor_tensor(ps[:, :, :], p2[:, :, :], p1[:, 8:12, :], Alu.mult)

                # ---- fused 5-layer sort (physical planes hold LOGMAP lanes) ----
                T1 = psrt.tile([128, 24, BL], f32, tag="T1", name="T1")
                nc.vector.tensor_tensor(T1[:, 0:6, :], a[:, 0:6, :], a[:, 12:18, :], Alu.min)
                nc.vector.tensor_tensor(T1[:, 12:18, :], a[:, 0:6, :], a[:, 12:18, :], Alu.max)
                nc.vector.tensor_tensor(T1[:, 6:12, :], a[:, 6:12, :], a[:, 18:24, :], Alu.min)
                nc.vector.tensor_tensor(T1[:, 18:24, :], a[:, 6:12, :], a[:, 18:24, :], Alu.max)
                # T1 planes = [pos0, pos1, pos2, pos5, pos3, pos4]
                T2 = psrt.tile([128, 24, BL], f32, tag="T2", name="T2")
                nc.vector.tensor_tensor(w2(T2, 1), g3(T1, 1), g3(T1, 2), Alu.min)
                nc.vector.tensor_tensor(w2(T2, 2), g3(T1, 1), g3(T1, 2), Alu.max)
                # T2 planes (1..4) = [pos1, pos2, pos3, pos4]; pos0 @ T1[0], pos5 @ T1[3]
                T3 = psrt.tile([128, 24, BL], f32, tag="T3", name="T3")
                nc.vector.tensor_tensor(pl(T3, 0), pl(T1, 0), pl(T2, 3), Alu.min)
                nc.vector.tensor_tensor(pl(T3, 4), pl(T1, 0), pl(T2, 3), Alu.max)
                nc.vector.tensor_tensor(pl(T3, 1), pl(T2, 2), pl(T1, 3), Alu.min)
                nc.vector.tensor_tensor(pl(T3, 5), pl(T2, 2), pl(T1, 3), Alu.max)
                nc.vector.tensor_copy(pl(T3, 2), pl(T2, 4))
                nc.vector.tensor_copy(pl(T3, 3), pl(T2, 1))
                # T3 planes = [pos0, pos2, pos4, pos1, pos3, pos5]
                T4 = psrt.tile([128, 24, BL], f32, tag="T4", name="T4")
                nc.vector.tensor_tensor(pl(T4, 0, 3), pl(T3, 0, 3), pl(T3, 3, 3), Alu.min)
                nc.vector.tensor_tensor(pl(T4, 3, 3), pl(T3, 0, 3), pl(T3, 3, 3), Alu.max)
                S13 = psrt.tile([128, 8, BL], f32, tag="S13", name="S13")
                nc.vector.tensor_tensor(S13[:, :, :], pl(T4, 3, 2), pl(T4, 1, 2), Alu.min)
                S24 = psrt.tile([128, 8, BL], f32, tag="S24", name="S24")
                nc.vector.tensor_tensor(S24[:, :, :], pl(T4, 3, 2), pl(T4, 1, 2), Alu.max)
                lanes = [pl(T4, 0), S13[:, 0:4, :], S24[:, 0:4, :],
                         S13[:, 4:8, :], S24[:, 4:8, :], pl(T4, 5)]

                # u_k = w_k s_k (Act, scaled copies); base tree on any
                u = []
                for kk in range(5):
                    uk = psm.tile([128, 4, BL], f32, tag=f"u{kk}", name=f"uk{kk}")
                    nc.scalar.activation(uk[:, :, :], lanes[kk], ActF.Copy, scale=w[kk])
                    u.append(uk)
                b01 = psm.tile([128, 4, BL], f32, tag="b01")
                nc.any.tensor_tensor(b01[:, :, :], u[0][:, :, :], u[1][:, :, :], Alu.add)
                b23 = psm.tile([128, 4, BL], f32, tag="b23")
                nc.any.tensor_tensor(b23[:, :, :], u[2][:, :, :], u[3][:, :, :], Alu.add)
                b03 = psm.tile([128, 4, BL], f32, tag="b03")
                nc.any.tensor_tensor(b03[:, :, :], b01[:, :, :], b23[:, :, :], Alu.add)
                base = psm.tile([128, 4, BL], f32, tag="base")
                nc.any.tensor_tensor(base[:, :, :], b03[:, :, :], u[4][:, :, :], Alu.add)

                # e_k = w_k (s_{k+1} - s_k): diff on DVE, scale on Act
                e = []
                for kk in range(5):
                    dk = psm.tile([128, 4, BL], f32, tag=f"d{kk}", name=f"dk{kk}")
                    nc.vector.tensor_tensor(dk[:, :, :], lanes[kk + 1], lanes[kk], Alu.subtract)
                    ek = psm.tile([128, 4, BL], f32, tag=f"e{kk}", name=f"ek{kk}")
                    nc.scalar.activation(ek[:, :, :], dk[:, :, :], ActF.Copy, scale=w[kk])
                    e.append(ek)

                a4 = a[:, :, :].rearrange("p (j m) b -> p j m b", j=DC)
                bshape = [128, DC, 4, BL]
                terms = []
                for kk in range(5):
                    cmp = pw.tile([128, 24, BL], f32, tag=f"cmp{kk}", name=f"cmp{kk}")
                    cmp4 = cmp[:, :, :].rearrange("p (j m) b -> p j m b", j=DC)
                    sk_b = lanes[kk].unsqueeze(1).broadcast_to(bshape)
                    nc.vector.tensor_tensor(cmp4, sk_b, a4, Alu.is_ge)
                    ek_b = e[kk][:, :, :].unsqueeze(1).broadcast_to(bshape)
                    nc.vector.tensor_tensor(cmp4, cmp4, ek_b, Alu.mult)
                    terms.append(cmp)
                t01 = pw.tile([128, 24, BL], f32, tag="t01")
                nc.vector.tensor_tensor(t01[:, :, :], terms[0][:, :, :], terms[1][:, :, :], Alu.add)
                t23 = pw.tile([128, 24, BL], f32, tag="t23")
                nc.vector.tensor_tensor(t23[:, :, :], terms[2][:, :, :], terms[3][:, :, :], Alu.add)
                t4b = pw.tile([128, 24, BL], f32, tag="t4b")
                t4b4 = t4b[:, :, :].rearrange("p (j m) b -> p j m b", j=DC)
                nc.vector.tensor_tensor(
                    t4b4, terms[4][:, :, :].rearrange("p (j m) b -> p j m b", j=DC),
                    base[:, :, :].unsqueeze(1).broadcast_to(bshape), Alu.add)
                t0123 = pw.tile([128, 24, BL], f32, tag="t0123")
                nc.vector.tensor_tensor(t0123[:, :, :], t01[:, :, :], t23[:, :, :], Alu.add)
                acc = pw.tile([128, 24, BL], f32, tag="acc")
                nc.vector.tensor_tensor(acc[:, 0:12, :], t0123[:, 0:12, :], t4b[:, 0:12, :], Alu.add)
                nc.vector.tensor_tensor(acc[:, 12:24, :], t0123[:, 12:24, :], t4b[:, 12:24, :], Alu.add)

                # sg_loo = sg * psign on gpsimd (off the DVE chain)
                sg_loo = pw.tile([128, 24, BL], f32, tag="sgloo")
                sgl4 = sg_loo[:, :, :].rearrange("p (j m) b -> p j m b", j=DC)
                sg4 = sg[:, :, :].rearrange("p (j m) b -> p j m b", j=DC)
                ps_b = ps[:, :, :].unsqueeze(1).broadcast_to(bshape)
                nc.gpsimd.tensor_tensor(sgl4, sg4, ps_b, Alu.mult)
                cv = pw.tile([128, 24, BL], f32, tag="cv", name="cv")
                nc.vector.tensor_tensor(cv[:, 0:12, :], acc[:, 0:12, :], sg_loo[:, 0:12, :], Alu.mult)
                nc.vector.tensor_tensor(cv[:, 12:24, :], acc[:, 12:24, :], sg_loo[:, 12:24, :], Alu.mult)
                if it < NUM_ITERS:
                    qm = pw.tile([128, 24, BL], f32, tag="qm", name="qm")
                    nc.gpsimd.tensor_tensor(qm[:, :, :], gc1[:, :, :], cv[:, :, :], Alu.subtract)

                # ---- split + colsum (two k-halves so the next gather can
                # start on the first half while the second is still summing) ----
                cv_tri = pw.tile([128, 24, 3, BL], bf16, tag="cvtri", name="cv_tri")
                split_tri(cv[:, 0:12, :], cv_tri[:, 0:12, :, :], 12)
                split_tri(cv[:, 12:24, :], cv_tri[:, 12:24, :, :], 12)
                cs_tri = [None] * 4
                for h in range(4):
                    cs_ps = do_colsum_half(cv_tri, h)
                    csh = pw.tile([128, 2, BL], f32, tag=f"csh{h}", name="csh")
                    nc.vector.tensor_copy(csh[:, :, :], cs_ps[:, :, 0, :])
                    csm = pw.tile([128, 2, BL], f32, tag=f"csm{h}", name="csm")
                    nc.vector.tensor_tensor(csm[:, :, :], csh[:, :, :], cs_ps[:, :, 1, :], Alu.add)
                    cs = pw.tile([128, 2, BL], f32, tag=f"cs{h}", name="cs")
                    nc.vector.tensor_tensor(cs[:, :, :], csm[:, :, :], cs_ps[:, :, 2, :], Alu.add)

                    so = pw.tile([128, 2, BL], f32, tag=f"so{h}", name="so")
                    nc.any.tensor_tensor(so[:, :, :], cs[:, :, :], c2[:, 2 * h:2 * h + 2, :], Alu.add)
                    nc.sync.dma_start(
                        out[it].rearrange("(nh p) b -> p nh b", p=128)[:, 2 * h:2 * h + 2, :],
                        so[:, :, :])

                    if it < NUM_ITERS:
                        tri = pw.tile([128, 2, 3, BL], bf16, tag=f"ttri{h}", name="ttri")
                        split_tri(cs[:, :, :], tri, 2)
                        cs_tri[h] = tri

    nc.compile()
    return nc


def _get_nc(row_cols, W1, W2, bit_w1, bit_w2):
    row_cols = np.asarray(row_cols)
    w = (np.asarray(W1, np.float32) @ np.asarray(W2, np.float32))[:, 0]
    sp1 = float(np.log1p(np.exp(np.asarray(bit_w1, np.float32)))[0])
    sp2 = float(np.log1p(np.exp(np.asarray(bit_w2, np.float32)))[0])
    key = (row_cols.tobytes(), w.tobytes(), sp1, sp2)
    if key not in _CACHE:
        import ml_dtypes
        assign, cols, present, gt, ct = _layout(row_cols)
        wg, wc = _weights(cols, gt, ct)
        f8 = ml_dtypes.float8_e4m3fn
        _CACHE[key] = (_build(cols, w, sp1, sp2, gt, ct),
                       np.ascontiguousarray(wg.reshape(128, -1).astype(f8)),
                       np.ascontiguousarray(wc.reshape(128, -1).astype(f8)),
                       cols, sp1)
    return _CACHE[key]


def _in_maps(inputs):
    soft = np.asarray(inputs["soft_input"], np.float32)
    nc, wg, wc, cols, sp1 = _get_nc(inputs["row_cols"], inputs["W1"], inputs["W2"],
                                    inputs["bit_w1"], inputs["bit_w2"])
    in_maps = []
    for c in range(NCORES):
        shard = soft[c * BL:(c + 1) * BL, :]  # [8, 1024]
        c1 = (shard * np.float32(sp1)).astype(np.float32)
        g = c1[:, cols.reshape(24, 128)]          # [8, 24, 128]
        g = np.ascontiguousarray(g.transpose(2, 1, 0).reshape(128, 24 * BL))
        in_maps.append({
            "soft_t": np.ascontiguousarray(shard.T),  # [1024, 8]
            "gc1": g.astype(np.float32),
            "wg": wg,
            "wc": wc,
        })
    return nc, in_maps


def kernel(**inputs):
    from concourse.bass_utils import run_bass_kernel_spmd

    nc, in_maps = _in_maps(inputs)
    res = run_bass_kernel_spmd(nc, in_maps, core_ids=list(range(NCORES)))

    full = np.empty((NUM_ITERS + 1, B, N), np.float32)
    for c in range(NCORES):
        o = res.results[c]["out"]  # [6, 1024, 8]
        full[:, c * BL:(c + 1) * BL, :] = o.transpose(0, 2, 1)
    return full

